# revision 13
# baseline (speedup 1.0000x reference)
"""Trainium2 Bass kernel for nn_ConvSelfAttentionModule (B=4, C=256, H=W=64).

Reference computation per image (xf = x reshaped to [C, N], N = H*W = 4096):
    q = wq @ xf + bq                       [128, N]
    k = wk @ xf + bk                       [128, N]
    v = wv @ xf + bv                       [256, N]
    s[m, n]   = sum_d q[d, m] k[d, n]      [N, N]
    attn      = softmax_n(s)
    af[c, n]  = sum_m v[c, m] attn[m, n]   [256, N]
    out = gamma * af + x

Sharding: 8 cores = 4 images x 2 m-chunks of M=2048 rows of the attention
matrix (columns pre-rolled per core so the chunk is always cols 0:2048; host
rolls back and sums the two partials per image, then adds x).

v2 design (all-bf16 matmuls, software-pipelined):
  - x is DMA'd f32 and converted to bf16 on DVE; all projections and the
    scores/af matmuls run bf16 (512-wide bf16 matmuls issue every ~216ns on
    HW vs ~335-430ns for f32r).
  - Per m-tile: scores h0 -> exp h0 (ACT, accum rowsum), scores h1 ->
    exp h1 (accum rowsum); DVE then folds gamma/rowsum into v[mt]
    immediately, unlocking af contributions for that tile.
  - af = v'T.T @ E accumulates in one PSUM tile per (h, c, mt-range-of-4)
    stage; stages are interleaved into the exp-bound sweep via a filler
    queue on the PE (the sweep is ACT-bound at ~2us/exp while the 4 scores
    matmuls take only ~0.9us). Stage partials are combined in SBUF (bf16)
    by DVE copies/adds, and DMA'd out per (h, c) as soon as complete.
"""

import collections

import numpy as np
import ml_dtypes

import concourse.bass as bass  # noqa: F401  (bass types via bacc/tile)
import concourse.tile as tile
from concourse import bacc, mybir
from concourse.bass_utils import run_bass_kernel_spmd

dt = mybir.dt

P = 128          # partitions / q,k channel dim
C = 256          # channels
N = 4096         # pixels per image
M = 2048         # per-core m-chunk
MT = M // P      # 16 m-tiles
B = 4
N_CORES = 8
EXP_SHIFT = -20.0  # constant subtracted inside exp; cancels in softmax
LAG = 3            # h1 exp of tile mt is issued after h0 exp of tile mt+LAG
QRANGES = [(0, 4), (4, 8), (8, 12), (12, 16)]  # af stage mt-ranges

_CACHE = {}


def build_nc():
    nc = bacc.Bacc("TRN2", target_bir_lowering=False, debug=False,
                   num_devices=N_CORES)
    f32, bf16 = dt.float32, dt.bfloat16
    AF = mybir.ActivationFunctionType

    x = nc.dram_tensor("x", [C, N], f32, kind="ExternalInput").ap()
    wqT = nc.dram_tensor("wqT", [C, P], bf16, kind="ExternalInput").ap()
    wkT = nc.dram_tensor("wkT", [C, P], bf16, kind="ExternalInput").ap()
    wvT = nc.dram_tensor("wvT", [C, C], bf16, kind="ExternalInput").ap()
    bq = nc.dram_tensor("bq", [P, 1], f32, kind="ExternalInput").ap()
    bk = nc.dram_tensor("bk", [P, 1], f32, kind="ExternalInput").ap()
    bv = nc.dram_tensor("bv", [1, C], f32, kind="ExternalInput").ap()
    out = nc.dram_tensor("out_part", [C, N], bf16, kind="ExternalOutput").ap()

    with tile.TileContext(nc) as tc:
        with (
            tc.tile_pool(name="consts", bufs=1) as consts,
            tc.tile_pool(name="xs", bufs=2) as xs,
            tc.tile_pool(name="xb", bufs=1) as xbp,
            tc.tile_pool(name="big", bufs=1) as big,
            tc.tile_pool(name="es", bufs=MT) as es,
            tc.tile_pool(name="afs", bufs=1) as afs,
            tc.tile_pool(name="ps_s", bufs=1, space="PSUM") as ps_s,
            tc.tile_pool(name="ps_a", bufs=1, space="PSUM") as ps_a,
        ):
            # ---- constant DMAs (HWDGE on sync for the early-needed ones) ----
            wk_t, wq_t = [], []
            for i in range(2):
                wki = consts.tile([P, P], bf16, name=f"wk{i}", tag=f"wk{i}")
                nc.sync.dma_start(out=wki, in_=wkT[i * P:(i + 1) * P, :])
                wk_t.append(wki)
            bk_t = consts.tile([P, 1], f32, name="bk_t", tag="bk_t")
            nc.sync.dma_start(out=bk_t, in_=bk)
            for i in range(2):
                wqi = consts.tile([P, P], bf16, name=f"wq{i}", tag=f"wq{i}")
                nc.sync.dma_start(out=wqi, in_=wqT[i * P:(i + 1) * P, :])
                wq_t.append(wqi)
            bq_t = consts.tile([P, 1], f32, name="bq_t", tag="bq_t")
            nc.sync.dma_start(out=bq_t, in_=bq)
            wv_t = []
            for i in range(2):
                wvi = consts.tile([P, C], bf16, name=f"wv{i}", tag=f"wv{i}")
                nc.gpsimd.dma_start(out=wvi, in_=wvT[i * P:(i + 1) * P, :])
                wv_t.append(wvi)
            bv_bc = consts.tile([P, C], f32, name="bv_bc", tag="bv_bc")
            nc.gpsimd.dma_start(out=bv_bc, in_=bv.to_broadcast((P, C)))
            shift_t = consts.tile([P, 1], f32, name="shift_t", tag="shift_t")
            nc.vector.memset(shift_t, EXP_SHIFT)

            # Dummy exp so the ACT function-table load happens during the
            # DMA prologue, not before the first real exp.
            warm_t = consts.tile([P, 1], f32, name="warm_t", tag="warm_t")
            nc.scalar.activation(warm_t, shift_t, AF.Exp, bias=shift_t[:, 0:1],
                                 scale=1.0)

            # 4 partial rowsums per m-tile: (h, n-half-of-1024)
            rs = consts.tile([P, MT, 4], f32, name="rs", tag="rs")
            rr = consts.tile([P, MT], f32, name="rr", tag="rr")

            k_sb = big.tile([P, N], bf16, name="k_sb", tag="k_sb")
            q_sb = big.tile([P, M], bf16, name="q_sb", tag="q_sb")
            v_sb = big.tile([P, MT, C], bf16, name="v_sb", tag="v_sb")

            # ---- x DMA (f32) + DVE convert to bf16 ----
            # g0 (cols 0:2048) on the ACT hwdge queue (idle in prologue) in
            # 1024-col sub-chunks alternating c-halves so the DVE converts
            # pipeline behind the transfers. g1 is issued from gpsimd whose
            # queue can block on the staging bufs for free.
            xg_b = []
            for g in range(2):
                eng = nc.scalar if g == 0 else nc.gpsimd
                x0 = xs.tile([P, M], f32, name=f"xf0_{g}", tag="xf")
                x1 = xs.tile([P, M], f32, name=f"xf1_{g}", tag="xf")
                b0 = xbp.tile([P, M], bf16, name=f"xb0_{g}", tag=f"xb0_{g}")
                b1 = xbp.tile([P, M], bf16, name=f"xb1_{g}", tag=f"xb1_{g}")
                for sub in range(2):
                    ssl = slice(sub * 1024, (sub + 1) * 1024)
                    gsl = slice(g * M + sub * 1024, g * M + (sub + 1) * 1024)
                    eng.dma_start(out=x0[:, ssl], in_=x[0:P, gsl])
                    eng.dma_start(out=x1[:, ssl], in_=x[P:C, gsl])
                    nc.vector.tensor_copy(b0[:, ssl], x0[:, ssl])
                    nc.vector.tensor_copy(b1[:, ssl], x1[:, ssl])
                xg_b.append((b0, b1))

            # ---- q/k projections (bf16), [P, 1024] ping-pong tiles ----
            # k0/q evacuate on ACT/DVE split (both idle in the prologue);
            # k1 (mid-sweep) evacuates on DVE so the ACT exp chain is never
            # head-of-line blocked behind a PE-gated evacuation.
            def proj_1024(wt, x0, x1, dst, bias_t, half, on_act):
                sp = ps_s.tile([P, 1024], f32, name=f"pj{id(wt)}_{half}",
                               tag="ps_s")
                for j in range(2):
                    sl = slice(half * 1024 + j * 512, half * 1024 + (j + 1) * 512)
                    psl = slice(j * 512, (j + 1) * 512)
                    nc.tensor.matmul(sp[:, psl], wt[0], x0[:, sl],
                                     start=True, stop=False)
                    nc.tensor.matmul(sp[:, psl], wt[1], x1[:, sl],
                                     start=False, stop=True)
                dsl = slice(half * 1024, (half + 1) * 1024)
                if on_act:
                    nc.scalar.activation(dst[:, dsl], sp, AF.Identity,
                                         bias=bias_t[:, 0:1], scale=1.0)
                else:
                    nc.vector.tensor_scalar_add(dst[:, dsl], sp,
                                                bias_t[:, 0:1])

            def k_group(g):
                x0, x1 = xg_b[g]
                dst = k_sb[:, g * M:(g + 1) * M]
                for half in range(2):
                    proj_1024(wk_t, x0, x1, dst, bk_t, half, on_act=(g == 0))

            k_group(0)
            x0, x1 = xg_b[0]
            for half in range(2):
                proj_1024(wq_t, x0, x1, q_sb, bq_t, half, on_act=False)

            # ---- PE filler queue ----
            # Items are (gate_key, closure). gate_key None = always ready;
            # otherwise the closure is only popped once `unlocked` contains
            # the key. Keys: ("k1",) after h0[3] issued; ("fold", mt) after
            # the v-fold for mt was issued on DVE.
            filler = collections.deque()
            unlocked = set()

            def drain(budget_mm):
                done = 0
                while filler and done < budget_mm:
                    gate, n_mm, fn = filler[0]
                    if gate is not None and gate not in unlocked:
                        break
                    filler.popleft()
                    fn()
                    done += max(n_mm, 1)

            # v projection chunks (PE filler, early)
            def v_chunk(ck):
                x0, x1 = xg_b[0]
                vp = ps_a.tile([P, 2, C], f32, name=f"vp{ck}", tag="ps_a")
                for i in range(2):
                    t = ck * 2 + i
                    xsl = slice(t * P, (t + 1) * P)
                    nc.tensor.matmul(vp[:, i], x0[:, xsl], wv_t[0],
                                     start=True, stop=False)
                    nc.tensor.matmul(vp[:, i], x1[:, xsl], wv_t[1],
                                     start=False, stop=True)
                for i in range(2):
                    t = ck * 2 + i
                    nc.vector.tensor_add(v_sb[:, t, :], vp[:, i], bv_bc)

            for ck in range(8):
                filler.append((None, 4, (lambda c=ck: v_chunk(c))))

            # ---- af stages ----
            e_tiles = []
            af_sb = []
            for h in range(2):
                for c in range(2):
                    t = afs.tile([P, M], bf16, name=f"af{h}{c}", tag=f"af{h}{c}")
                    af_sb.append(t)

            def af_stage(h, c, q):
                lo, hi = QRANGES[q]
                ap_t = ps_a.tile([P, M], f32, name=f"ap{h}{c}{q}", tag="ps_a")
                for mt in range(lo, hi):
                    lhs = v_sb[:, mt, c * P:(c + 1) * P]
                    for j in range(4):
                        nc.tensor.matmul(
                            ap_t[:, j * 512:(j + 1) * 512], lhs,
                            e_tiles[mt][:, h, j * 512:(j + 1) * 512],
                            start=(mt == lo), stop=(mt == hi - 1))
                dst = af_sb[h * 2 + c]
                if q == 0:
                    # all evacs on DVE: an ACT copy here would head-of-line
                    # block the exp chain behind PE-gated stage matmuls
                    nc.vector.tensor_copy(dst, ap_t)
                else:
                    nc.vector.tensor_add(dst, ap_t, dst)
                if q == len(QRANGES) - 1:
                    nc.sync.dma_start(
                        out=out[c * P:(c + 1) * P, h * M:(h + 1) * M],
                        in_=dst)

            for q in range(len(QRANGES)):
                gate = ("fold", QRANGES[q][1] - 1)
                for h in range(2):
                    for c in range(2):
                        filler.append(
                            (gate, 16, (lambda hh=h, cc=c, qq=q:
                                        af_stage(hh, cc, qq))))

            # ---- the sweep ----
            # scores/exp run at [P, 1024] granularity through a 2-deep
            # PSUM ping-pong (2 banks each) so the ACT exp chain never
            # waits on the PE: exp(slot i) runs while the PE fills slot
            # i+1. The af stage accumulator keeps the remaining 4 banks.
            def scores(mt, h):
                q_l = q_sb[:, mt * P:(mt + 1) * P]
                if h == 0:
                    e_t = es.tile([P, 2, M], bf16, name=f"e{mt}", tag="e")
                    e_tiles.append(e_t)
                e_t = e_tiles[mt]
                for half in range(2):
                    sp = ps_s.tile([P, 1024], f32, name=f"sp{mt}_{h}_{half}",
                                   tag="ps_s")
                    for j in (2 * half, 2 * half + 1):
                        k_l = k_sb[:, h * M + j * 512:h * M + (j + 1) * 512]
                        nc.tensor.matmul(sp[:, (j % 2) * 512:(j % 2 + 1) * 512],
                                         q_l, k_l, start=True, stop=True)
                    nc.scalar.activation(
                        e_t[:, h, half * 1024:(half + 1) * 1024], sp,
                        AF.Exp, bias=shift_t[:, 0:1], scale=1.0,
                        accum_out=rs[:, mt, 2 * h + half:2 * h + half + 1])
                    drain(2)

            def fold(mt):
                nc.vector.reduce_sum(rr[:, mt:mt + 1], rs[:, mt, :],
                                     axis=mybir.AxisListType.X)
                nc.vector.reciprocal(rr[:, mt:mt + 1], rr[:, mt:mt + 1])
                nc.vector.tensor_scalar_mul(v_sb[:, mt, :], v_sb[:, mt, :],
                                            rr[:, mt:mt + 1])
                unlocked.add(("fold", mt))

            # interleaved (mt, h) order with h1 lagging LAG tiles behind h0
            order = []
            for mt in range(MT + LAG):
                if mt < MT:
                    order.append((mt, 0))
                if mt >= LAG:
                    order.append((mt - LAG, 1))

            for i, (mt, h) in enumerate(order):
                if (mt, h) == (0, 1):
                    # k1 must be emitted before any h1 scores matmul reads
                    # k_sb[:, M:N]; by now the x-g1 bf16 convert is queued.
                    k_group(1)
                scores(mt, h)
                if h == 1:
                    fold(mt)
                drain(8)

            # drain whatever is left (af tail stages)
            drain(10 ** 9)

    nc.compile()
    return nc


def _get_nc():
    if "nc" not in _CACHE:
        _CACHE["nc"] = build_nc()
    return _CACHE["nc"]


def build_in_maps(x, wq, bq, wk, bk, wv, bv, gamma):
    x = np.asarray(x, np.float32)
    g = float(np.asarray(gamma).reshape(-1)[0])
    bf = ml_dtypes.bfloat16
    wqT = np.ascontiguousarray(np.asarray(wq, np.float32).T.astype(bf))
    wkT = np.ascontiguousarray(np.asarray(wk, np.float32).T.astype(bf))
    wvT = np.ascontiguousarray((g * np.asarray(wv, np.float32)).T.astype(bf))
    bq2 = np.ascontiguousarray(np.asarray(bq, np.float32).reshape(P, 1))
    bk2 = np.ascontiguousarray(np.asarray(bk, np.float32).reshape(P, 1))
    bv2 = np.ascontiguousarray((g * np.asarray(bv, np.float32)).reshape(1, C))
    xf = x.reshape(B, C, N)
    in_maps = []
    for core in range(N_CORES):
        b, half = core // 2, core % 2
        xc = xf[b] if half == 0 else np.roll(xf[b], -M, axis=1)
        in_maps.append(dict(x=np.ascontiguousarray(xc), wqT=wqT, wkT=wkT,
                            wvT=wvT, bq=bq2, bk=bk2, bv=bv2))
    return in_maps


def assemble(results, x):
    x = np.asarray(x, np.float32)
    af = np.zeros((B, C, N), np.float32)
    for core in range(N_CORES):
        b, half = core // 2, core % 2
        part = np.asarray(results[core]["out_part"]).astype(np.float32)
        af[b] += part if half == 0 else np.roll(part, M, axis=1)
    return (af.reshape(x.shape) + x).astype(np.float32)


def kernel(x, wq, bq, wk, bk, wv, bv, gamma):
    nc = _get_nc()
    in_maps = build_in_maps(x, wq, bq, wk, bk, wv, bv, gamma)
    res = run_bass_kernel_spmd(nc, in_maps, core_ids=list(range(N_CORES)))
    return assemble(res.results, x)


# revision 16
# speedup vs baseline: 1.4735x; 1.4735x over previous
"""Trainium2 Bass kernel for nn_ConvSelfAttentionModule (B=4, C=256, H=W=64).

Reference computation per image (xf = x reshaped to [C, N], N = H*W = 4096):
    q = wq @ xf + bq                       [128, N]
    k = wk @ xf + bk                       [128, N]
    v = wv @ xf + bv                       [256, N]
    s[m, n]   = sum_d q[d, m] k[d, n]      [N, N]
    attn      = softmax_n(s)
    af[c, n]  = sum_m v[c, m] attn[m, n]   [256, N]
    out = gamma * af + x

Sharding: 8 cores = 4 images x 2 m-chunks of M=2048 rows of the attention
matrix (columns pre-rolled per core so the chunk is always cols 0:2048; host
rolls back and sums the two partials per image, then adds x).

v2 design (all-bf16 matmuls, software-pipelined):
  - x is DMA'd f32 and converted to bf16 on DVE; all projections and the
    scores/af matmuls run bf16 (512-wide bf16 matmuls issue every ~216ns on
    HW vs ~335-430ns for f32r).
  - Per m-tile: scores h0 -> exp h0 (ACT, accum rowsum), scores h1 ->
    exp h1 (accum rowsum); DVE then folds gamma/rowsum into v[mt]
    immediately, unlocking af contributions for that tile.
  - af = v'T.T @ E accumulates in one PSUM tile per (h, c, mt-range-of-4)
    stage; stages are interleaved into the exp-bound sweep via a filler
    queue on the PE (the sweep is ACT-bound at ~2us/exp while the 4 scores
    matmuls take only ~0.9us). Stage partials are combined in SBUF (bf16)
    by DVE copies/adds, and DMA'd out per (h, c) as soon as complete.
"""

import collections

import numpy as np
import ml_dtypes

import concourse.bass as bass  # noqa: F401  (bass types via bacc/tile)
import concourse.tile as tile
from concourse import bacc, mybir
from concourse.bass_utils import run_bass_kernel_spmd

dt = mybir.dt

P = 128          # partitions / q,k channel dim
C = 256          # channels
N = 4096         # pixels per image
M = 2048         # per-core m-chunk
MT = M // P      # 16 m-tiles
B = 4
N_CORES = 8
EXP_SHIFT = -20.0  # constant subtracted inside exp; cancels in softmax
LAG = 3            # h1 exp of tile mt is issued after h0 exp of tile mt+LAG
QRANGES = [(0, 4), (4, 8), (8, 12), (12, 16)]  # af stage mt-ranges

_CACHE = {}


def build_nc():
    nc = bacc.Bacc("TRN2", target_bir_lowering=False, debug=False,
                   num_devices=N_CORES)
    f32, bf16 = dt.float32, dt.bfloat16
    AF = mybir.ActivationFunctionType

    x = nc.dram_tensor("x", [C, N], f32, kind="ExternalInput").ap()
    wqT = nc.dram_tensor("wqT", [C, P], bf16, kind="ExternalInput").ap()
    wkT = nc.dram_tensor("wkT", [C, P], bf16, kind="ExternalInput").ap()
    wvT = nc.dram_tensor("wvT", [C, C], bf16, kind="ExternalInput").ap()
    bq = nc.dram_tensor("bq", [P, 1], f32, kind="ExternalInput").ap()
    bk = nc.dram_tensor("bk", [P, 1], f32, kind="ExternalInput").ap()
    bv = nc.dram_tensor("bv", [1, C], f32, kind="ExternalInput").ap()
    out = nc.dram_tensor("out_part", [C, N], bf16, kind="ExternalOutput").ap()

    with tile.TileContext(nc) as tc:
        with (
            tc.tile_pool(name="consts", bufs=1) as consts,
            tc.tile_pool(name="xs", bufs=2) as xs,
            tc.tile_pool(name="xb", bufs=1) as xbp,
            tc.tile_pool(name="big", bufs=1) as big,
            tc.tile_pool(name="es", bufs=MT) as es,
            tc.tile_pool(name="afs", bufs=1) as afs,
            tc.tile_pool(name="ps_s", bufs=3, space="PSUM") as ps_s,
            tc.tile_pool(name="ps_a", bufs=1, space="PSUM") as ps_a,
        ):
            # ---- constant DMAs (HWDGE on sync for the early-needed ones) ----
            wk_t, wq_t = [], []
            for i in range(2):
                wki = consts.tile([P, P], bf16, name=f"wk{i}", tag=f"wk{i}")
                nc.sync.dma_start(out=wki, in_=wkT[i * P:(i + 1) * P, :])
                wk_t.append(wki)
            bk_t = consts.tile([P, 1], f32, name="bk_t", tag="bk_t")
            nc.sync.dma_start(out=bk_t, in_=bk)
            for i in range(2):
                wqi = consts.tile([P, P], bf16, name=f"wq{i}", tag=f"wq{i}")
                nc.sync.dma_start(out=wqi, in_=wqT[i * P:(i + 1) * P, :])
                wq_t.append(wqi)
            bq_t = consts.tile([P, 1], f32, name="bq_t", tag="bq_t")
            nc.sync.dma_start(out=bq_t, in_=bq)
            wv_t = []
            for i in range(2):
                wvi = consts.tile([P, C], bf16, name=f"wv{i}", tag=f"wv{i}")
                nc.gpsimd.dma_start(out=wvi, in_=wvT[i * P:(i + 1) * P, :])
                wv_t.append(wvi)
            bv_bc = consts.tile([P, C], f32, name="bv_bc", tag="bv_bc")
            nc.gpsimd.dma_start(out=bv_bc, in_=bv.to_broadcast((P, C)))
            shift_t = consts.tile([P, 1], f32, name="shift_t", tag="shift_t")
            nc.vector.memset(shift_t, EXP_SHIFT)

            # Dummy exp so the ACT function-table load happens during the
            # DMA prologue, not before the first real exp.
            warm_t = consts.tile([P, 1], f32, name="warm_t", tag="warm_t")
            nc.scalar.activation(warm_t, shift_t, AF.Exp, bias=shift_t[:, 0:1],
                                 scale=1.0)

            # 4 partial rowsums per m-tile: (h, n-half-of-1024)
            rs = consts.tile([P, MT, 4], f32, name="rs", tag="rs")
            rr = consts.tile([P, MT], f32, name="rr", tag="rr")

            k_sb = big.tile([P, N], bf16, name="k_sb", tag="k_sb")
            q_sb = big.tile([P, M], bf16, name="q_sb", tag="q_sb")
            v_sb = big.tile([P, MT, C], bf16, name="v_sb", tag="v_sb")

            # ---- x DMA (f32) + DVE convert to bf16 ----
            # g0 (cols 0:2048) on the ACT hwdge queue (idle in prologue) in
            # 1024-col sub-chunks alternating c-halves so the DVE converts
            # pipeline behind the transfers. g1 is issued from gpsimd whose
            # queue can block on the staging bufs for free.
            xg_b = []
            for g in range(2):
                eng = nc.scalar if g == 0 else nc.gpsimd
                x0 = xs.tile([P, M], f32, name=f"xf0_{g}", tag="xf")
                x1 = xs.tile([P, M], f32, name=f"xf1_{g}", tag="xf")
                b0 = xbp.tile([P, M], bf16, name=f"xb0_{g}", tag=f"xb0_{g}")
                b1 = xbp.tile([P, M], bf16, name=f"xb1_{g}", tag=f"xb1_{g}")
                for sub in range(2):
                    ssl = slice(sub * 1024, (sub + 1) * 1024)
                    gsl = slice(g * M + sub * 1024, g * M + (sub + 1) * 1024)
                    eng.dma_start(out=x0[:, ssl], in_=x[0:P, gsl])
                    eng.dma_start(out=x1[:, ssl], in_=x[P:C, gsl])
                    nc.vector.tensor_copy(b0[:, ssl], x0[:, ssl])
                    nc.vector.tensor_copy(b1[:, ssl], x1[:, ssl])
                xg_b.append((b0, b1))

            # ---- q/k projections (bf16), [P, 1024] ping-pong tiles ----
            # k0/q evacuate on ACT/DVE split (both idle in the prologue);
            # k1 (mid-sweep) evacuates on DVE so the ACT exp chain is never
            # head-of-line blocked behind a PE-gated evacuation.
            def proj_1024(wt, x0, x1, dst, bias_t, half, on_act):
                sp = ps_s.tile([P, 1024], f32, name=f"pj{id(wt)}_{half}",
                               tag="ps_s")
                for j in range(2):
                    sl = slice(half * 1024 + j * 512, half * 1024 + (j + 1) * 512)
                    psl = slice(j * 512, (j + 1) * 512)
                    nc.tensor.matmul(sp[:, psl], wt[0], x0[:, sl],
                                     start=True, stop=False)
                    nc.tensor.matmul(sp[:, psl], wt[1], x1[:, sl],
                                     start=False, stop=True)
                dsl = slice(half * 1024, (half + 1) * 1024)
                if on_act:
                    nc.scalar.activation(dst[:, dsl], sp, AF.Identity,
                                         bias=bias_t[:, 0:1], scale=1.0)
                else:
                    nc.vector.tensor_scalar_add(dst[:, dsl], sp,
                                                bias_t[:, 0:1])

            def k_group(g):
                x0, x1 = xg_b[g]
                dst = k_sb[:, g * M:(g + 1) * M]
                for half in range(2):
                    proj_1024(wk_t, x0, x1, dst, bk_t, half, on_act=(g == 0))

            k_group(0)
            x0, x1 = xg_b[0]
            for half in range(2):
                proj_1024(wq_t, x0, x1, q_sb, bq_t, half, on_act=False)

            # ---- PE filler queue ----
            # Items are (gate_key, closure). gate_key None = always ready;
            # otherwise the closure is only popped once `unlocked` contains
            # the key. Keys: ("k1",) after h0[3] issued; ("fold", mt) after
            # the v-fold for mt was issued on DVE.
            filler = collections.deque()
            unlocked = set()

            def drain(budget_mm):
                done = 0
                while filler and done < budget_mm:
                    gate, n_mm, fn = filler[0]
                    if gate is not None and gate not in unlocked:
                        break
                    filler.popleft()
                    fn()
                    done += max(n_mm, 1)

            # v projection chunks (PE filler, early)
            def v_chunk(ck):
                x0, x1 = xg_b[0]
                vp = ps_a.tile([P, 2, C], f32, name=f"vp{ck}", tag="ps_a")
                for i in range(2):
                    t = ck * 2 + i
                    xsl = slice(t * P, (t + 1) * P)
                    nc.tensor.matmul(vp[:, i], x0[:, xsl], wv_t[0],
                                     start=True, stop=False)
                    nc.tensor.matmul(vp[:, i], x1[:, xsl], wv_t[1],
                                     start=False, stop=True)
                for i in range(2):
                    t = ck * 2 + i
                    nc.vector.tensor_add(v_sb[:, t, :], vp[:, i], bv_bc)

            for ck in range(8):
                filler.append((None, 4, (lambda c=ck: v_chunk(c))))

            # ---- af stages ----
            e_tiles = []
            af_sb = []
            for h in range(2):
                for c in range(2):
                    t = afs.tile([P, M], bf16, name=f"af{h}{c}", tag=f"af{h}{c}")
                    af_sb.append(t)

            def af_stage(h, c, nh, q):
                # [P, 1024] accumulator (2 PSUM banks) over mt in QRANGES[q]
                lo, hi = QRANGES[q]
                ap_t = ps_a.tile([P, 1024], f32, name=f"ap{h}{c}{nh}{q}",
                                 tag="ps_a")
                for mt in range(lo, hi):
                    lhs = v_sb[:, mt, c * P:(c + 1) * P]
                    for j in (2 * nh, 2 * nh + 1):
                        nc.tensor.matmul(
                            ap_t[:, (j % 2) * 512:(j % 2 + 1) * 512], lhs,
                            e_tiles[mt][:, h, j * 512:(j + 1) * 512],
                            start=(mt == lo), stop=(mt == hi - 1))
                dst = af_sb[h * 2 + c]
                dsl = slice(nh * 1024, (nh + 1) * 1024)
                # all evacs on DVE: an ACT copy here would head-of-line
                # block the exp chain behind PE-gated stage matmuls
                if q == 0:
                    nc.vector.tensor_copy(dst[:, dsl], ap_t)
                else:
                    nc.vector.tensor_add(dst[:, dsl], ap_t, dst[:, dsl])
                if q == len(QRANGES) - 1:
                    nc.sync.dma_start(
                        out=out[c * P:(c + 1) * P,
                                h * M + nh * 1024:h * M + (nh + 1) * 1024],
                        in_=dst[:, dsl])

            for q in range(len(QRANGES)):
                # one-tile margin on the gate so stage matmuls never reach
                # the PE queue head before their folds have executed
                gate = ("fold", min(QRANGES[q][1], MT - 1))
                for h in range(2):
                    for c in range(2):
                        for nh in range(2):
                            filler.append(
                                (gate, 8, (lambda hh=h, cc=c, nn=nh, qq=q:
                                           af_stage(hh, cc, nn, qq))))

            # ---- the sweep (software-pipelined emission) ----
            # The PE queue always holds the scores matmuls for exp slot
            # i+2 BEFORE any filler for slot i, so the ACT exp chain runs
            # back-to-back while af/v/k1 filler consumes the PE slack
            # behind the lookahead. sp pool is a 3-deep [P, 1024] ping-pong.
            pending_sp = {}

            def scores_mms(mt, h, half):
                if h == 0 and half == 0:
                    e_t = es.tile([P, 2, M], bf16, name=f"e{mt}", tag="e")
                    e_tiles.append(e_t)
                if (mt, h, half) == (0, 1, 0):
                    # k1 must be emitted before any h1 scores matmul reads
                    # k_sb[:, M:N]; by now the x-g1 bf16 convert is queued.
                    k_group(1)
                q_l = q_sb[:, mt * P:(mt + 1) * P]
                sp = ps_s.tile([P, 1024], f32, name=f"sp{mt}_{h}_{half}",
                               tag="ps_s")
                for j in (2 * half, 2 * half + 1):
                    k_l = k_sb[:, h * M + j * 512:h * M + (j + 1) * 512]
                    nc.tensor.matmul(sp[:, (j % 2) * 512:(j % 2 + 1) * 512],
                                     q_l, k_l, start=True, stop=True)
                pending_sp[(mt, h, half)] = sp

            def exp_slot(mt, h, half):
                sp = pending_sp.pop((mt, h, half))
                e_t = e_tiles[mt]
                nc.scalar.activation(
                    e_t[:, h, half * 1024:(half + 1) * 1024], sp,
                    AF.Exp, bias=shift_t[:, 0:1], scale=1.0,
                    accum_out=rs[:, mt, 2 * h + half:2 * h + half + 1])

            def fold(mt):
                nc.vector.reduce_sum(rr[:, mt:mt + 1], rs[:, mt, :],
                                     axis=mybir.AxisListType.X)
                nc.vector.reciprocal(rr[:, mt:mt + 1], rr[:, mt:mt + 1])
                nc.vector.tensor_scalar_mul(v_sb[:, mt, :], v_sb[:, mt, :],
                                            rr[:, mt:mt + 1])
                unlocked.add(("fold", mt))

            # interleaved (mt, h) order with h1 lagging LAG tiles behind h0
            order = []
            for mt in range(MT + LAG):
                if mt < MT:
                    order.append((mt, 0))
                if mt >= LAG:
                    order.append((mt - LAG, 1))
            slots = [(mt, h, half) for (mt, h) in order for half in range(2)]

            scores_mms(*slots[0])
            scores_mms(*slots[1])
            for idx, slot in enumerate(slots):
                exp_slot(*slot)
                mt, h, half = slot
                if h == 1 and half == 1:
                    fold(mt)
                if idx + 2 < len(slots):
                    scores_mms(*slots[idx + 2])
                drain(6)

            # drain whatever is left (af tail stages)
            drain(10 ** 9)

    nc.compile()
    return nc


def _get_nc():
    if "nc" not in _CACHE:
        _CACHE["nc"] = build_nc()
    return _CACHE["nc"]


def build_in_maps(x, wq, bq, wk, bk, wv, bv, gamma):
    x = np.asarray(x, np.float32)
    g = float(np.asarray(gamma).reshape(-1)[0])
    bf = ml_dtypes.bfloat16
    wqT = np.ascontiguousarray(np.asarray(wq, np.float32).T.astype(bf))
    wkT = np.ascontiguousarray(np.asarray(wk, np.float32).T.astype(bf))
    wvT = np.ascontiguousarray((g * np.asarray(wv, np.float32)).T.astype(bf))
    bq2 = np.ascontiguousarray(np.asarray(bq, np.float32).reshape(P, 1))
    bk2 = np.ascontiguousarray(np.asarray(bk, np.float32).reshape(P, 1))
    bv2 = np.ascontiguousarray((g * np.asarray(bv, np.float32)).reshape(1, C))
    xf = x.reshape(B, C, N)
    in_maps = []
    for core in range(N_CORES):
        b, half = core // 2, core % 2
        xc = xf[b] if half == 0 else np.roll(xf[b], -M, axis=1)
        in_maps.append(dict(x=np.ascontiguousarray(xc), wqT=wqT, wkT=wkT,
                            wvT=wvT, bq=bq2, bk=bk2, bv=bv2))
    return in_maps


def assemble(results, x):
    x = np.asarray(x, np.float32)
    af = np.zeros((B, C, N), np.float32)
    for core in range(N_CORES):
        b, half = core // 2, core % 2
        part = np.asarray(results[core]["out_part"]).astype(np.float32)
        af[b] += part if half == 0 else np.roll(part, M, axis=1)
    return (af.reshape(x.shape) + x).astype(np.float32)


def kernel(x, wq, bq, wk, bk, wv, bv, gamma):
    nc = _get_nc()
    in_maps = build_in_maps(x, wq, bq, wk, bk, wv, bv, gamma)
    res = run_bass_kernel_spmd(nc, in_maps, core_ids=list(range(N_CORES)))
    return assemble(res.results, x)


# revision 19
# speedup vs baseline: 1.8042x; 1.2245x over previous
"""Trainium2 Bass kernel for nn_ConvSelfAttentionModule (B=4, C=256, H=W=64).

Reference computation per image (xf = x reshaped to [C, N], N = H*W = 4096):
    q = wq @ xf + bq                       [128, N]
    k = wk @ xf + bk                       [128, N]
    v = wv @ xf + bv                       [256, N]
    s[m, n]   = sum_d q[d, m] k[d, n]      [N, N]
    attn      = softmax_n(s)
    af[c, n]  = sum_m v[c, m] attn[m, n]   [256, N]
    out = gamma * af + x

Sharding: 8 cores = 4 images x 2 m-chunks of M=2048 rows of the attention
matrix (columns pre-rolled per core so the chunk is always cols 0:2048; host
rolls back and sums the two partials per image, then adds x).

v2 design (all-bf16 matmuls, software-pipelined):
  - x is DMA'd f32 and converted to bf16 on DVE; all projections and the
    scores/af matmuls run bf16 (512-wide bf16 matmuls issue every ~216ns on
    HW vs ~335-430ns for f32r).
  - Per m-tile: scores h0 -> exp h0 (ACT, accum rowsum), scores h1 ->
    exp h1 (accum rowsum); DVE then folds gamma/rowsum into v[mt]
    immediately, unlocking af contributions for that tile.
  - af = v'T.T @ E accumulates in one PSUM tile per (h, c, mt-range-of-4)
    stage; stages are interleaved into the exp-bound sweep via a filler
    queue on the PE (the sweep is ACT-bound at ~2us/exp while the 4 scores
    matmuls take only ~0.9us). Stage partials are combined in SBUF (bf16)
    by DVE copies/adds, and DMA'd out per (h, c) as soon as complete.
"""

import collections

import numpy as np
import ml_dtypes

import concourse.bass as bass  # noqa: F401  (bass types via bacc/tile)
import concourse.tile as tile
from concourse import bacc, mybir
from concourse.bass_utils import run_bass_kernel_spmd

dt = mybir.dt

P = 128          # partitions / q,k channel dim
C = 256          # channels
N = 4096         # pixels per image
M = 2048         # per-core m-chunk
MT = M // P      # 16 m-tiles
B = 4
N_CORES = 8
EXP_SHIFT = -20.0  # constant subtracted inside exp; cancels in softmax
LAG = 3            # h1 exp of tile mt is issued after h0 exp of tile mt+LAG
QRANGES = [(0, 4), (4, 8), (8, 12), (12, 16)]  # af stage mt-ranges

_CACHE = {}


def build_nc():
    nc = bacc.Bacc("TRN2", target_bir_lowering=False, debug=False,
                   num_devices=N_CORES)
    f32, bf16 = dt.float32, dt.bfloat16
    AF = mybir.ActivationFunctionType

    x = nc.dram_tensor("x", [C, N], f32, kind="ExternalInput").ap()
    wqT = nc.dram_tensor("wqT", [C, P], bf16, kind="ExternalInput").ap()
    wkT = nc.dram_tensor("wkT", [C, P], bf16, kind="ExternalInput").ap()
    wvT = nc.dram_tensor("wvT", [C, C], bf16, kind="ExternalInput").ap()
    bq = nc.dram_tensor("bq", [P, 1], f32, kind="ExternalInput").ap()
    bk = nc.dram_tensor("bk", [P, 1], f32, kind="ExternalInput").ap()
    bv = nc.dram_tensor("bv", [1, C], f32, kind="ExternalInput").ap()
    out = nc.dram_tensor("out_part", [C, N], bf16, kind="ExternalOutput").ap()

    with tile.TileContext(nc) as tc:
        with (
            tc.tile_pool(name="consts", bufs=1) as consts,
            tc.tile_pool(name="xs", bufs=2) as xs,
            tc.tile_pool(name="xb", bufs=1) as xbp,
            tc.tile_pool(name="big", bufs=1) as big,
            tc.tile_pool(name="es", bufs=MT) as es,
            tc.tile_pool(name="afs", bufs=1) as afs,
            tc.tile_pool(name="ps_s", bufs=2, space="PSUM") as ps_s,
            tc.tile_pool(name="ps_a", bufs=2, space="PSUM") as ps_a,
        ):
            # ---- constant DMAs (HWDGE on sync for the early-needed ones) ----
            wk_t, wq_t = [], []
            for i in range(2):
                wki = consts.tile([P, P], bf16, name=f"wk{i}", tag=f"wk{i}")
                nc.sync.dma_start(out=wki, in_=wkT[i * P:(i + 1) * P, :])
                wk_t.append(wki)
            bk_t = consts.tile([P, 1], f32, name="bk_t", tag="bk_t")
            nc.sync.dma_start(out=bk_t, in_=bk)
            for i in range(2):
                wqi = consts.tile([P, P], bf16, name=f"wq{i}", tag=f"wq{i}")
                nc.sync.dma_start(out=wqi, in_=wqT[i * P:(i + 1) * P, :])
                wq_t.append(wqi)
            bq_t = consts.tile([P, 1], f32, name="bq_t", tag="bq_t")
            nc.sync.dma_start(out=bq_t, in_=bq)
            wv_t = []
            for i in range(2):
                wvi = consts.tile([P, C], bf16, name=f"wv{i}", tag=f"wv{i}")
                nc.gpsimd.dma_start(out=wvi, in_=wvT[i * P:(i + 1) * P, :])
                wv_t.append(wvi)
            bv_bc = consts.tile([P, C], f32, name="bv_bc", tag="bv_bc")
            nc.gpsimd.dma_start(out=bv_bc, in_=bv.to_broadcast((P, C)))
            shift_t = consts.tile([P, 1], f32, name="shift_t", tag="shift_t")
            nc.vector.memset(shift_t, EXP_SHIFT)

            # Dummy exp so the ACT function-table load happens during the
            # DMA prologue, not before the first real exp.
            warm_t = consts.tile([P, 1], f32, name="warm_t", tag="warm_t")
            nc.scalar.activation(warm_t, shift_t, AF.Exp, bias=shift_t[:, 0:1],
                                 scale=1.0)

            # 4 partial rowsums per m-tile: (h, n-half-of-1024)
            rs = consts.tile([P, MT, 4], f32, name="rs", tag="rs")
            rr = consts.tile([P, MT], f32, name="rr", tag="rr")

            k_sb = big.tile([P, N], bf16, name="k_sb", tag="k_sb")
            q_sb = big.tile([P, M], bf16, name="q_sb", tag="q_sb")
            v_sb = big.tile([P, MT, C], bf16, name="v_sb", tag="v_sb")

            # ---- x DMA (f32) + DVE convert to bf16 ----
            # g0 (cols 0:2048) on the ACT hwdge queue (idle in prologue) in
            # 1024-col sub-chunks alternating c-halves so the DVE converts
            # pipeline behind the transfers. g1 is issued from gpsimd whose
            # queue can block on the staging bufs for free.
            xg_b = []

            def load_x_group(g):
                eng = nc.scalar if g == 0 else nc.gpsimd
                x0 = xs.tile([P, M], f32, name=f"xf0_{g}", tag="xf")
                x1 = xs.tile([P, M], f32, name=f"xf1_{g}", tag="xf")
                b0 = xbp.tile([P, M], bf16, name=f"xb0_{g}", tag=f"xb0_{g}")
                b1 = xbp.tile([P, M], bf16, name=f"xb1_{g}", tag=f"xb1_{g}")
                for sub in range(2):
                    ssl = slice(sub * 1024, (sub + 1) * 1024)
                    gsl = slice(g * M + sub * 1024, g * M + (sub + 1) * 1024)
                    eng.dma_start(out=x0[:, ssl], in_=x[0:P, gsl])
                    eng.dma_start(out=x1[:, ssl], in_=x[P:C, gsl])
                    nc.vector.tensor_copy(b0[:, ssl], x0[:, ssl])
                    nc.vector.tensor_copy(b1[:, ssl], x1[:, ssl])
                xg_b.append((b0, b1))

            # g1 is loaded AFTER the q/k0 projections are emitted so its
            # DVE converts never head-of-line block the q evacuation.
            load_x_group(0)

            # ---- q/k projections (bf16), [P, 1024] ping-pong tiles ----
            # k0/q evacuate on ACT/DVE split (both idle in the prologue);
            # k1 (mid-sweep) evacuates on DVE so the ACT exp chain is never
            # head-of-line blocked behind a PE-gated evacuation.
            def proj_1024(wt, x0, x1, dst, bias_t, half, on_act):
                sp = ps_s.tile([P, 1024], f32, name=f"pj{id(wt)}_{half}",
                               tag="ps_s")
                for j in range(2):
                    sl = slice(half * 1024 + j * 512, half * 1024 + (j + 1) * 512)
                    psl = slice(j * 512, (j + 1) * 512)
                    nc.tensor.matmul(sp[:, psl], wt[0], x0[:, sl],
                                     start=True, stop=False)
                    nc.tensor.matmul(sp[:, psl], wt[1], x1[:, sl],
                                     start=False, stop=True)
                dsl = slice(half * 1024, (half + 1) * 1024)
                if on_act:
                    nc.scalar.activation(dst[:, dsl], sp, AF.Identity,
                                         bias=bias_t[:, 0:1], scale=1.0)
                else:
                    nc.vector.tensor_scalar_add(dst[:, dsl], sp,
                                                bias_t[:, 0:1])

            def k_group(g):
                x0, x1 = xg_b[g]
                dst = k_sb[:, g * M:(g + 1) * M]
                for half in range(2):
                    proj_1024(wk_t, x0, x1, dst, bk_t, half, on_act=(g == 0))

            k_group(0)
            x0, x1 = xg_b[0]
            for half in range(2):
                # q evac on ACT too: it's idle in the prologue, and the DVE
                # will be busy with the g1 converts emitted next
                proj_1024(wq_t, x0, x1, q_sb, bq_t, half, on_act=True)

            load_x_group(1)

            # ---- PE filler queue ----
            # Items are (gate_key, closure). gate_key None = always ready;
            # otherwise the closure is only popped once `unlocked` contains
            # the key. Keys: ("k1",) after h0[3] issued; ("fold", mt) after
            # the v-fold for mt was issued on DVE.
            filler = collections.deque()
            unlocked = set()

            def drain(budget_mm):
                done = 0
                while filler and done < budget_mm:
                    gate, n_mm, fn = filler[0]
                    if gate is not None and gate not in unlocked:
                        break
                    filler.popleft()
                    fn()
                    done += max(n_mm, 1)

            # v projection chunks (PE filler, early)
            def v_chunk(ck):
                x0, x1 = xg_b[0]
                vp = ps_a.tile([P, 2, C], f32, name=f"vp{ck}", tag="ps_a")
                for i in range(2):
                    t = ck * 2 + i
                    xsl = slice(t * P, (t + 1) * P)
                    nc.tensor.matmul(vp[:, i], x0[:, xsl], wv_t[0],
                                     start=True, stop=False)
                    nc.tensor.matmul(vp[:, i], x1[:, xsl], wv_t[1],
                                     start=False, stop=True)
                for i in range(2):
                    t = ck * 2 + i
                    nc.vector.tensor_add(v_sb[:, t, :], vp[:, i], bv_bc)

            for ck in range(8):
                filler.append((None, 4, (lambda c=ck: v_chunk(c))))

            # ---- af stages ----
            e_tiles = []
            af_sb = []
            for h in range(2):
                for c in range(2):
                    t = afs.tile([P, M], bf16, name=f"af{h}{c}", tag=f"af{h}{c}")
                    af_sb.append(t)

            def af_stage(h, c, nh, q):
                # [P, 1024] accumulator (2 PSUM banks) over mt in QRANGES[q]
                lo, hi = QRANGES[q]
                ap_t = ps_a.tile([P, 1024], f32, name=f"ap{h}{c}{nh}{q}",
                                 tag="ps_a")
                for mt in range(lo, hi):
                    lhs = v_sb[:, mt, c * P:(c + 1) * P]
                    for j in (2 * nh, 2 * nh + 1):
                        nc.tensor.matmul(
                            ap_t[:, (j % 2) * 512:(j % 2 + 1) * 512], lhs,
                            e_tiles[mt][:, h, j * 512:(j + 1) * 512],
                            start=(mt == lo), stop=(mt == hi - 1))
                dst = af_sb[h * 2 + c]
                dsl = slice(nh * 1024, (nh + 1) * 1024)
                # all evacs on DVE: an ACT copy here would head-of-line
                # block the exp chain behind PE-gated stage matmuls
                if q == 0:
                    nc.vector.tensor_copy(dst[:, dsl], ap_t)
                else:
                    nc.vector.tensor_add(dst[:, dsl], ap_t, dst[:, dsl])
                if q == len(QRANGES) - 1:
                    nc.sync.dma_start(
                        out=out[c * P:(c + 1) * P,
                                h * M + nh * 1024:h * M + (nh + 1) * 1024],
                        in_=dst[:, dsl])

            for q in range(len(QRANGES)):
                # one-tile margin on the gate so stage matmuls never reach
                # the PE queue head before their folds have executed
                gate = ("fold", min(QRANGES[q][1], MT - 1))
                for h in range(2):
                    for c in range(2):
                        for nh in range(2):
                            filler.append(
                                (gate, 8, (lambda hh=h, cc=c, nn=nh, qq=q:
                                           af_stage(hh, cc, nn, qq))))

            # ---- the sweep (software-pipelined emission) ----
            # The PE queue always holds the scores matmuls for exp slot
            # i+2 BEFORE any filler for slot i, so the ACT exp chain runs
            # back-to-back while af/v/k1 filler consumes the PE slack
            # behind the lookahead. sp pool is a 3-deep [P, 1024] ping-pong.
            pending_sp = {}

            def scores_mms(mt, h, half):
                if h == 0 and half == 0:
                    e_t = es.tile([P, 2, M], bf16, name=f"e{mt}", tag="e")
                    e_tiles.append(e_t)
                if (mt, h, half) == (0, 1, 0):
                    # k1 must be emitted before any h1 scores matmul reads
                    # k_sb[:, M:N]; by now the x-g1 bf16 convert is queued.
                    k_group(1)
                q_l = q_sb[:, mt * P:(mt + 1) * P]
                sp = ps_s.tile([P, 1024], f32, name=f"sp{mt}_{h}_{half}",
                               tag="ps_s")
                for j in (2 * half, 2 * half + 1):
                    k_l = k_sb[:, h * M + j * 512:h * M + (j + 1) * 512]
                    nc.tensor.matmul(sp[:, (j % 2) * 512:(j % 2 + 1) * 512],
                                     q_l, k_l, start=True, stop=True)
                pending_sp[(mt, h, half)] = sp

            def exp_slot(mt, h, half):
                sp = pending_sp.pop((mt, h, half))
                e_t = e_tiles[mt]
                nc.scalar.activation(
                    e_t[:, h, half * 1024:(half + 1) * 1024], sp,
                    AF.Exp, bias=shift_t[:, 0:1], scale=1.0,
                    accum_out=rs[:, mt, 2 * h + half:2 * h + half + 1])

            def fold(mt):
                nc.vector.reduce_sum(rr[:, mt:mt + 1], rs[:, mt, :],
                                     axis=mybir.AxisListType.X)
                nc.vector.reciprocal(rr[:, mt:mt + 1], rr[:, mt:mt + 1])
                nc.vector.tensor_scalar_mul(v_sb[:, mt, :], v_sb[:, mt, :],
                                            rr[:, mt:mt + 1])
                unlocked.add(("fold", mt))

            # interleaved (mt, h) order with h1 lagging LAG tiles behind h0
            order = []
            for mt in range(MT + LAG):
                if mt < MT:
                    order.append((mt, 0))
                if mt >= LAG:
                    order.append((mt - LAG, 1))
            slots = [(mt, h, half) for (mt, h) in order for half in range(2)]

            scores_mms(*slots[0])
            scores_mms(*slots[1])
            for idx, slot in enumerate(slots):
                exp_slot(*slot)
                mt, h, half = slot
                if h == 1 and half == 1:
                    fold(mt)
                if idx + 2 < len(slots):
                    scores_mms(*slots[idx + 2])
                drain(6)

            # drain whatever is left (af tail stages)
            drain(10 ** 9)

    nc.compile()
    return nc


def _get_nc():
    if "nc" not in _CACHE:
        _CACHE["nc"] = build_nc()
    return _CACHE["nc"]


def build_in_maps(x, wq, bq, wk, bk, wv, bv, gamma):
    x = np.asarray(x, np.float32)
    g = float(np.asarray(gamma).reshape(-1)[0])
    bf = ml_dtypes.bfloat16
    wqT = np.ascontiguousarray(np.asarray(wq, np.float32).T.astype(bf))
    wkT = np.ascontiguousarray(np.asarray(wk, np.float32).T.astype(bf))
    wvT = np.ascontiguousarray((g * np.asarray(wv, np.float32)).T.astype(bf))
    bq2 = np.ascontiguousarray(np.asarray(bq, np.float32).reshape(P, 1))
    bk2 = np.ascontiguousarray(np.asarray(bk, np.float32).reshape(P, 1))
    bv2 = np.ascontiguousarray((g * np.asarray(bv, np.float32)).reshape(1, C))
    xf = x.reshape(B, C, N)
    in_maps = []
    for core in range(N_CORES):
        b, half = core // 2, core % 2
        xc = xf[b] if half == 0 else np.roll(xf[b], -M, axis=1)
        in_maps.append(dict(x=np.ascontiguousarray(xc), wqT=wqT, wkT=wkT,
                            wvT=wvT, bq=bq2, bk=bk2, bv=bv2))
    return in_maps


def assemble(results, x):
    x = np.asarray(x, np.float32)
    af = np.zeros((B, C, N), np.float32)
    for core in range(N_CORES):
        b, half = core // 2, core % 2
        part = np.asarray(results[core]["out_part"]).astype(np.float32)
        af[b] += part if half == 0 else np.roll(part, M, axis=1)
    return (af.reshape(x.shape) + x).astype(np.float32)


def kernel(x, wq, bq, wk, bk, wv, bv, gamma):
    nc = _get_nc()
    in_maps = build_in_maps(x, wq, bq, wk, bk, wv, bv, gamma)
    res = run_bass_kernel_spmd(nc, in_maps, core_ids=list(range(N_CORES)))
    return assemble(res.results, x)
